# revision 1
# baseline (speedup 1.0000x reference)
"""Trainium2 Bass kernel for nn_Attention (sparse_attention, T=3).

Math (per batch row b, derived from the reference):
    zq = z[:, :3*2048].reshape(B, 3, D)   (q and v source)
    zk = z[:, 3*2048:].reshape(B, 3, D)
    query = zq @ wq.T + bq ; key = zk @ wk.T + bk
    scores[t,s] = query[t].key[s]/sqrt(D)
               = (zq[t] @ (wq.T @ wk) @ zk[s] + zq[t].(wq.T @ bk)
                  + (bq @ wk).zk[s] + bq.bk) / sqrt(D)
    strictly-lower entries of scores are replaced by 0 before softmax
    p = softmax(scores, axis=-1); w[s] = sum_t p[t,s]  (sum_s w[s] == 3)
    y = (sum_s w[s]*zq[s]) @ wv.T @ wo.T + 3*bv @ wo.T + 3*bo

So per core (data-parallel shard of B):
    M = wq.T @ wk (PE, natural layouts), a = wq.T @ bk, r = bq @ wk, kap = bq.bk
    G~ = zq @ M + r  (PE, needs zq tiles transposed on-chip)
    scores via DVE tensor_tensor_reduce dots + PE matvec for zq.a
    softmax (ACT exp) -> w -> zv = sum_s w_s zq_s (DVE)
    uT = wv @ zv.T + 3bv ; y = (uT.T @ wo.T) + 3bo  (PE, weights transposed
    on-chip via PE transpose)
All matmuls in bf16 (PSUM f32 accumulate); dots/softmax in f32/bf16 mix.
Verified numerically: L2 rel err ~4e-3 vs f32 reference.
"""

import sys

sys.path.insert(0, "/opt/trn_rl_repo")

import numpy as np
from concourse import bacc, bass, masks, mybir, tile
from concourse.bass_utils import run_bass_kernel_spmd

F32 = mybir.dt.float32
BF16 = mybir.dt.bfloat16
ADD = mybir.AluOpType.add
MULT = mybir.AluOpType.mult

B = 8192
D = 2048
T = 3
NCORES = 8
DC = D // 128      # 16 d-chunks
EC = D // 512      # 4 e-chunks (512-wide psum banks)
SQD = 1.0 / float(np.sqrt(np.float32(D)))


def emit(tc, aps, b_loc, stages=("p1", "p2", "p3", "p4")):
    nc = tc.nc
    z, wq, bq, wk, bk, wv, bv, wo, bo, out = (
        aps["z"], aps["wq"], aps["bq"], aps["wk"], aps["bk"],
        aps["wv"], aps["bv"], aps["wo"], aps["bo"], aps["out"],
    )
    NB = b_loc // 128

    const = tc.alloc_tile_pool(name="const", bufs=1)
    persist = tc.alloc_tile_pool(name="persist", bufs=1)

    ident = const.tile([128, 128], BF16)
    masks.make_identity(nc, ident[:])

    # --- biases ---
    # column layouts: col[p, c] = vec[c*128 + p]
    bq_col = const.tile([128, DC], F32)
    bk_col = const.tile([128, DC], F32)
    bv3_col = const.tile([128, DC], F32)
    nc.sync.dma_start(bq_col[:], bq.rearrange("(c p) -> p c", p=128))
    nc.sync.dma_start(bk_col[:], bk.rearrange("(c p) -> p c", p=128))
    nc.sync.dma_start(bv3_col[:], bv.rearrange("(c p) -> p c", p=128))
    nc.vector.tensor_scalar(bv3_col[:], bv3_col[:], 3.0, None, op0=MULT)
    bq_colbf = const.tile([128, DC], BF16)
    bk_colbf = const.tile([128, DC], BF16)
    nc.vector.tensor_copy(bq_colbf[:], bq_col[:])
    nc.vector.tensor_copy(bk_colbf[:], bk_col[:])

    # small persistent SBUF tensors (big ones get phase-scoped pools below)
    a_rep = persist.tile([128, D], BF16)           # a broadcast to partitions
    r_rowp = persist.tile([1, D], BF16)            # r as a single row
    ones_row = persist.tile([1, 128], BF16)        # rank-1 lhsT for r-add
    kap_col = persist.tile([128, 1], F32)          # kap/sqrt(D) per-partition

    m_pool = tc.alloc_tile_pool(name="m_pool", bufs=1, side="right")  # P1..P2
    m_bf = m_pool.tile([128, DC, D], BF16)         # M[d, e], partition = d%128

    # ---------------- Phase 1: M = wq.T @ wk, a = wq.T @ bk, r = bq @ wk ----
    with (
        tc.tile_pool(name="p1_wq", bufs=1) as p_wq,
        tc.tile_pool(name="p1_io", bufs=2) as p_io,
        tc.tile_pool(name="p1_wk", bufs=2) as p_wk,
        tc.tile_pool(name="p1_sm", bufs=2) as p_sm,
        tc.tile_pool(name="p1_psum", bufs=4, space="PSUM") as pp_m,
        tc.tile_pool(name="p1_psr", bufs=2, space="PSUM") as pp_r,
    ):
        wq_bf = p_wq.tile([128, DC, D], BF16)
        for n in range(DC):
            wq_f = p_io.tile([128, D], F32, tag="wload")
            nc.sync.dma_start(wq_f[:], wq[n * 128:(n + 1) * 128, :])
            if n % 2 == 0:
                nc.scalar.activation(wq_bf[:, n, :], wq_f[:],
                                     mybir.ActivationFunctionType.Copy)
            else:
                nc.vector.tensor_copy(wq_bf[:, n, :], wq_f[:])

        nc.vector.memset(ones_row[:], 1.0)
        for e in range(EC):
            wk_bf = p_wk.tile([128, DC, 512], BF16)
            for n in range(DC):
                wk_f = p_io.tile([128, 512], F32, tag="wkload", bufs=3)
                nc.sync.dma_start(
                    wk_f[:], wk[n * 128:(n + 1) * 128, e * 512:(e + 1) * 512])
                nc.scalar.activation(wk_bf[:, n, :], wk_f[:],
                                     mybir.ActivationFunctionType.Copy)
            for d in range(DC):
                ps = pp_m.tile([128, 512], F32)
                for n in range(DC):
                    nc.tensor.matmul(
                        ps[:], wq_bf[:, n, d * 128:(d + 1) * 128],
                        wk_bf[:, n, :], start=(n == 0), stop=(n == DC - 1))
                nc.vector.tensor_copy(m_bf[:, d, e * 512:(e + 1) * 512], ps[:])
            # r[e-slice] = bq @ wk[:, e-slice]
            ps_r = pp_r.tile([1, 512], F32)
            for n in range(DC):
                nc.tensor.matmul(ps_r[:], bq_colbf[:, n:n + 1], wk_bf[:, n, :],
                                 start=(n == 0), stop=(n == DC - 1))
            nc.vector.tensor_copy(r_rowp[:, e * 512:(e + 1) * 512], ps_r[:])

        # a = wq.T @ bk  as a row, then broadcast to all partitions
        a_row = p_sm.tile([1, D], BF16, bufs=1)
        for e in range(EC):
            ps_a = pp_r.tile([1, 512], F32, tag="psa")
            for n in range(DC):
                nc.tensor.matmul(ps_a[:], bk_colbf[:, n:n + 1],
                                 wq_bf[:, n, e * 512:(e + 1) * 512],
                                 start=(n == 0), stop=(n == DC - 1))
            nc.vector.tensor_copy(a_row[:, e * 512:(e + 1) * 512], ps_a[:])
        nc.gpsimd.partition_broadcast(a_rep[:], a_row[:])
        ps_k = pp_r.tile([1, 1], F32, tag="psa")
        for n in range(DC):
            nc.tensor.matmul(ps_k[:], bq_colbf[:, n:n + 1], bk_colbf[:, n:n + 1],
                             start=(n == 0), stop=(n == DC - 1))
        kap_row = p_sm.tile([1, 1], F32, bufs=1)
        nc.vector.tensor_copy(kap_row[:], ps_k[:])
        nc.gpsimd.partition_broadcast(kap_col[:], kap_row[:])
        nc.vector.tensor_scalar(kap_col[:], kap_col[:], SQD, None, op0=MULT)

    if not any(t.startswith("p2") for t in stages):
        m_pool.release()
        persist.release()
        const.release()
        return

    # ---------------- Phase 2: per b-tile scores/softmax/zv ----------------
    # Software-pipelined: section B (DVE/ACT dots+softmax+zv of tile ib-1)
    # is emitted before sections A/C (PE transposes + G matmuls of tile ib),
    # so the vector chain of one tile runs under the PE matmuls of the next.
    zvT_dram = nc.dram_tensor("zvT_dram", [DC, 128, b_loc], BF16).ap()
    with (
        tc.tile_pool(name="p2_io", bufs=2) as p_io,
        tc.tile_pool(name="p2_zq", bufs=1) as p_zq,
        tc.tile_pool(name="p2_g", bufs=1) as p_g,
        tc.tile_pool(name="p2_sc", bufs=1) as p_sc,
        tc.tile_pool(name="p2_pst", bufs=3, space="PSUM") as pp_t,
        tc.tile_pool(name="p2_psg", bufs=5, space="PSUM") as pp_g,
    ):
        EXP = mybir.ActivationFunctionType.Exp
        CPY = mybir.ActivationFunctionType.Copy

        def sec_a(ib):
            """loads + casts (ACT) + zq transposes for tile ib"""
            r0 = ib * 128
            st = {}
            st["zq_bf"] = p_zq.tile([128, T, D], BF16, tag="zqbf", bufs=2, name="zq_bf")
            for t in range(T):
                zq_f = p_io.tile([128, D], F32, tag="zf", bufs=3, name="zq_f")
                nc.sync.dma_start(
                    zq_f[:], z[r0:r0 + 128, t * D:(t + 1) * D])
                nc.scalar.activation(st["zq_bf"][:, t, :], zq_f[:], CPY)
            st["zk_bf"] = p_zq.tile([128, T, D], BF16, tag="zkbf", bufs=2, name="zk_bf")
            for s in range(T):
                zk_f = p_io.tile([128, D], F32, tag="zf", bufs=3, name="zk_f")
                nc.sync.dma_start(
                    zk_f[:], z[r0:r0 + 128, (T + s) * D:(T + s + 1) * D])
                # fold the 1/sqrt(D) score scale into the k cast
                nc.scalar.activation(st["zk_bf"][:, s, :], zk_f[:], CPY,
                                     scale=SQD)
            st["zqT"] = p_zq.tile([128, T, DC, 128], BF16, tag="zqT", bufs=1, name="zqT")
            for t in range(T):
                for dg in range(DC // 8):
                    ps = pp_t.tile([128, 8, 128], BF16)
                    for j in range(8):
                        d = dg * 8 + j
                        nc.tensor.matmul(
                            ps[:, j, :],
                            st["zq_bf"][:, t, d * 128:(d + 1) * 128],
                            ident[:], is_transpose=True)
                    nc.scalar.activation(
                        st["zqT"][:, t, dg * 8:(dg + 1) * 8, :], ps[:], CPY)
            return st

        def dots_t(ib, st, t, sraw):
            """score dot products for one query row t (DVE muls + mixed reduce)"""
            act_set = {0, 2, 4, 6} if ib < NB - 1 else {0, 1, 3, 4, 6, 7}
            gt = st["gt"]
            for s in range(T):
                scr = p_io.tile([128, D], BF16, tag="scr", bufs=3)
                nc.vector.tensor_tensor(scr[:], gt[:, t, :],
                                        st["zk_bf"][:, s, :], op=MULT)
                if 3 * s + t in act_set:
                    scr2 = p_io.tile([128, D], BF16, tag="scr", bufs=3)
                    nc.scalar.activation(scr2[:], scr[:], CPY,
                                         accum_out=sraw[:, t, s:s + 1])
                else:
                    nc.vector.tensor_reduce(sraw[:, t, s:s + 1], scr[:],
                                            axis=mybir.AxisListType.X,
                                            op=ADD)

        def sec_c(ib, st, fuse_dots=False):
            """zq transposes + G = zq @ M + r and tvec, interleaved per t"""
            gt = p_g.tile([128, T, D], BF16, tag="gt", bufs=1)
            st["gt"] = gt
            if fuse_dots:
                st["sraw"] = p_sc.tile([128, T, T], F32, tag="sraw", bufs=2, name="sraw")
            for t in range(T):
                for e in range(EC):
                    ps = pp_g.tile([128, 512], F32)
                    # rank-1 r-add folded into the PSUM accumulation
                    nc.tensor.matmul(ps[:], ones_row[:],
                                     r_rowp[:, e * 512:(e + 1) * 512],
                                     start=True, stop=False)
                    for d in range(DC):
                        nc.tensor.matmul(
                            ps[:], st["zqT"][:, t, d, :],
                            m_bf[:, d, e * 512:(e + 1) * 512],
                            start=False, stop=(d == DC - 1))
                    nc.scalar.activation(gt[:, t, e * 512:(e + 1) * 512],
                                         ps[:], CPY)
                if fuse_dots:
                    dots_t(ib, st, t, st["sraw"])

        def sec_b(ib, st):
            """scores dots + softmax + zv for tile ib (DVE/ACT only)"""
            tvec = p_sc.tile([128, T], F32, tag="tvec", bufs=1)
            traw = p_sc.tile([128, T], F32, tag="traw", bufs=1)
            for t in range(T):
                scr = p_io.tile([128, D], BF16, tag="scr", bufs=3)
                nc.vector.tensor_tensor(scr[:], st["zq_bf"][:, t, :], a_rep[:],
                                        op=MULT)
                if ib == NB - 1:
                    scr2 = p_io.tile([128, D], BF16, tag="scr", bufs=3)
                    nc.scalar.activation(scr2[:], scr[:], CPY,
                                         accum_out=traw[:, t:t + 1])
                else:
                    nc.vector.tensor_reduce(traw[:, t:t + 1], scr[:],
                                            axis=mybir.AxisListType.X, op=ADD)
            for t in range(T):
                nc.vector.tensor_scalar(tvec[:, t:t + 1], traw[:, t:t + 1],
                                        SQD, kap_col[:], op0=MULT, op1=ADD)
            if "sraw" in st:
                sraw = st["sraw"]
            else:
                sraw = p_sc.tile([128, T, T], F32, tag="sraw", bufs=2)
                for t in range(T):
                    dots_t(ib, st, t, sraw)
            # softmax; exp(score + tvec[t]) with masked entries = exp(0) = 1
            p_un = p_sc.tile([128, T, T], F32, tag="p_un", bufs=1)
            nc.scalar.activation(p_un[:, 0, :], sraw[:, 0, :], EXP,
                                 bias=tvec[:, 0:1])
            nc.scalar.activation(p_un[:, 1, 1:], sraw[:, 1, 1:], EXP,
                                 bias=tvec[:, 1:2])
            nc.scalar.activation(p_un[:, 2, 2:], sraw[:, 2, 2:], EXP,
                                 bias=tvec[:, 2:3])
            nc.vector.memset(p_un[:, 1, 0:1], 1.0)
            nc.vector.memset(p_un[:, 2, 0:2], 1.0)
            rsum = p_sc.tile([128, T], F32, tag="rsum", bufs=1)
            nc.vector.tensor_reduce(rsum[:], p_un[:],
                                    axis=mybir.AxisListType.X, op=ADD)
            rinv = p_sc.tile([128, T], F32, tag="rinv", bufs=1)
            nc.vector.reciprocal(rinv[:], rsum[:])
            pn = p_sc.tile([128, T, T], F32, tag="pn", bufs=1)
            for t in range(T):
                nc.vector.tensor_scalar(pn[:, t, :], p_un[:, t, :],
                                        rinv[:, t:t + 1], None, op0=MULT)
            ws = p_sc.tile([128, T], F32, tag="ws", bufs=1)
            nc.vector.tensor_reduce(ws[:], pn.rearrange("p t s -> p s t"),
                                    axis=mybir.AxisListType.X, op=ADD)
            # zv = sum_s ws[s] * zq[s]   (bf16; muls split DVE/ACT)
            zv_bf = p_sc.tile([128, D], BF16, tag="zv", bufs=2)
            zv_t1 = p_io.tile([128, D], BF16, tag="scr", bufs=3, name="zv_t1")
            nc.vector.tensor_scalar(zv_bf[:], st["zq_bf"][:, 0, :], ws[:, 0:1],
                                    None, op0=MULT)
            nc.scalar.activation(zv_t1[:], st["zq_bf"][:, 1, :], CPY,
                                 scale=ws[:, 1:2])
            nc.vector.tensor_tensor(zv_bf[:], zv_bf[:], zv_t1[:], op=ADD)
            nc.scalar.activation(zv_t1[:], st["zq_bf"][:, 2, :], CPY,
                                 scale=ws[:, 2:3])
            nc.vector.tensor_tensor(zv_bf[:], zv_bf[:], zv_t1[:], op=ADD)
            st["zv"] = zv_bf

        def sec_d(ib, st):
            """transpose zv and spill zv^T[d, b] to DRAM"""
            r0 = ib * 128
            stg = p_g.tile([128, DC, 128], BF16, tag="zvstage", bufs=2)
            for dg in range(DC // 8):
                ps = pp_t.tile([128, 8, 128], BF16)
                for j in range(8):
                    d = dg * 8 + j
                    nc.tensor.matmul(ps[:, j, :],
                                     st["zv"][:, d * 128:(d + 1) * 128],
                                     ident[:], is_transpose=True)
                nc.vector.tensor_copy(stg[:, dg * 8:(dg + 1) * 8, :], ps[:])
            nc.sync.dma_start(
                zvT_dram[:, :, r0:r0 + 128].rearrange("c p b -> p c b"),
                stg[:])

        state = [None] * NB
        for ib in range(NB):
            state[ib] = sec_a(ib)
            sec_c(ib, state[ib])
            if ib > 0:
                sec_b(ib - 1, state[ib - 1])
                sec_d(ib - 1, state[ib - 1])
        sec_b(NB - 1, state[NB - 1])
        sec_d(NB - 1, state[NB - 1])

    m_pool.release()

    if "p3" not in stages:
        m_pool.release()
        persist.release()
        const.release()
        return

    # ---------------- Phase 3: uT = wv @ zv.T + 3bv ------------------------
    uT_pool = tc.alloc_tile_pool(name="uT_pool", bufs=1)    # P3..P4
    uT = uT_pool.tile([128, DC, b_loc], BF16)      # u^T[n, b]
    wvT_pool = tc.alloc_tile_pool(name="wvT_pool", bufs=1, side="right")
    with (
        tc.tile_pool(name="p3_io", bufs=2) as p_io,
        tc.tile_pool(name="p3_pst", bufs=3, space="PSUM") as pp_t3,
        tc.tile_pool(name="p3_psu", bufs=5, space="PSUM") as pp_u,
    ):
        wvT = wvT_pool.tile([128, DC, D], BF16)
        for n in range(DC):
            wv_f = p_io.tile([128, D], F32, tag="wvf")
            nc.sync.dma_start(wv_f[:], wv[n * 128:(n + 1) * 128, :])
            wv_b = p_io.tile([128, D], BF16, tag="wvb")
            nc.scalar.activation(wv_b[:], wv_f[:],
                                 mybir.ActivationFunctionType.Copy)
            for dg in range(DC // 8):
                ps = pp_t3.tile([128, 8, 128], BF16)
                for j in range(8):
                    d = dg * 8 + j
                    nc.tensor.matmul(ps[:, j, :],
                                     wv_b[:, d * 128:(d + 1) * 128],
                                     ident[:], is_transpose=True)
                nc.vector.tensor_copy(
                    wvT[:, dg * 8:(dg + 1) * 8, n * 128:(n + 1) * 128]
                    .rearrange("p d b -> p d b"), ps[:])
        bw = min(512, b_loc)
        for h in range(b_loc // bw):
            zvh = p_io.tile([128, DC, bw], BF16, tag="zvh", bufs=2)
            nc.sync.dma_start(
                zvh[:],
                zvT_dram[:, :, h * bw:(h + 1) * bw].rearrange("c p b -> p c b"))
            for n in range(DC):
                ps = pp_u.tile([128, bw], F32)
                for d in range(DC):
                    nc.tensor.matmul(
                        ps[:], wvT[:, d, n * 128:(n + 1) * 128],
                        zvh[:, d, :],
                        start=(d == 0), stop=(d == DC - 1))
                nc.vector.tensor_scalar(uT[:, n, h * bw:(h + 1) * bw], ps[:],
                                        bv3_col[:, n:n + 1], None, op0=ADD)
    wvT_pool.release()

    if "p4" not in stages:
        uT_pool.release()
        persist.release()
        const.release()
        return

    # ---------------- Phase 4: y = uT.T @ wo.T + 3bo -----------------------
    with (
        tc.tile_pool(name="p4_io", bufs=2) as p_io,
        tc.tile_pool(name="p4_woT", bufs=1) as p_woT,
        tc.tile_pool(name="p4_y", bufs=2) as p_y,
        tc.tile_pool(name="p4_pst", bufs=3, space="PSUM") as pp_t4,
        tc.tile_pool(name="p4_psy", bufs=5, space="PSUM") as pp_y,
    ):
        bo3_row = p_woT.tile([1, D], F32)
        nc.sync.dma_start(bo3_row[:], bo[None, :])
        nc.vector.tensor_scalar(bo3_row[:], bo3_row[:], 3.0, None, op0=MULT)
        bo3_rep = p_woT.tile([128, D], F32)
        nc.gpsimd.partition_broadcast(bo3_rep[:], bo3_row[:])
        woT = p_woT.tile([128, DC, D], BF16)   # wo^T[n, g], partition = n%128
        for g in range(DC):
            wo_f = p_io.tile([128, D], F32, tag="wof")
            nc.sync.dma_start(wo_f[:], wo[g * 128:(g + 1) * 128, :])
            wo_b = p_io.tile([128, D], BF16, tag="wob")
            nc.scalar.activation(wo_b[:], wo_f[:],
                                 mybir.ActivationFunctionType.Copy)
            for n in range(DC):
                ps = pp_t4.tile([128, 128], BF16)
                nc.tensor.matmul(ps[:], wo_b[:, n * 128:(n + 1) * 128],
                                 ident[:], is_transpose=True)
                nc.vector.tensor_copy(woT[:, n, g * 128:(g + 1) * 128], ps[:])
        for ib in range(NB):
            y_sb = p_y.tile([128, D], F32)
            for e in range(EC):
                ps = pp_y.tile([128, 512], F32)
                for n in range(DC):
                    nc.tensor.matmul(
                        ps[:], uT[:, n, ib * 128:(ib + 1) * 128],
                        woT[:, n, e * 512:(e + 1) * 512],
                        start=(n == 0), stop=(n == DC - 1))
                nc.vector.tensor_tensor(y_sb[:, e * 512:(e + 1) * 512], ps[:],
                                        bo3_rep[:, e * 512:(e + 1) * 512],
                                        op=ADD)
            nc.sync.dma_start(out[ib * 128:(ib + 1) * 128, :], y_sb[:])

    uT_pool.release()
    persist.release()
    const.release()


def build_nc(b_loc, stages=("p1", "p2", "p3", "p4")):
    nc = bacc.Bacc("TRN2", target_bir_lowering=False, debug=False,
                   num_devices=NCORES)
    aps = {}
    aps["z"] = nc.dram_tensor("z", [b_loc, 2 * T * D], F32,
                              kind="ExternalInput").ap()
    for w in ("wq", "wk", "wv", "wo"):
        aps[w] = nc.dram_tensor(w, [D, D], F32, kind="ExternalInput").ap()
    for b_ in ("bq", "bk", "bv", "bo"):
        aps[b_] = nc.dram_tensor(b_, [D], F32, kind="ExternalInput").ap()
    aps["out"] = nc.dram_tensor("out", [b_loc, D], F32,
                                kind="ExternalOutput").ap()
    with tile.TileContext(nc) as tc:
        emit(tc, aps, b_loc, stages)
    nc.compile()
    return nc


_CACHE = {}


def _get_nc(b_loc):
    if b_loc not in _CACHE:
        _CACHE[b_loc] = build_nc(b_loc)
    return _CACHE[b_loc]


def kernel(**inputs):
    arrs = {k: np.ascontiguousarray(np.asarray(v, dtype=np.float32))
            for k, v in inputs.items()}
    b_loc = B // NCORES
    nc = _get_nc(b_loc)
    in_maps = []
    for c in range(NCORES):
        m = {k: arrs[k] for k in ("wq", "bq", "wk", "bk", "wv", "bv",
                                  "wo", "bo")}
        m["z"] = arrs["z"][c * b_loc:(c + 1) * b_loc]
        in_maps.append(m)
    res = run_bass_kernel_spmd(nc, in_maps, core_ids=list(range(NCORES)))
    return np.concatenate([r["out"] for r in res.results], axis=0)



# revision 7
# speedup vs baseline: 1.2742x; 1.2742x over previous
"""Trainium2 Bass kernel for nn_Attention (sparse_attention, T=3) — v2.

Math (per batch row b, derived from the reference):
    zq = z[:, :3*2048].reshape(B, 3, D)   (q and v source)
    zk = z[:, 3*2048:].reshape(B, 3, D)
    scores[t,s] = (zq[t] @ (wq.T @ wk) @ zk[s] + zq[t].(wq.T @ bk)
                   + (bq @ wk).zk[s] + bq.bk) / sqrt(D)
    strictly-lower entries of scores are replaced by 0 before softmax
    p = softmax(scores, axis=-1); w[s] = sum_t p[t,s]  (sum_s w[s] == 3)
    y = (sum_s w[s]*zq[s]) @ (wv.T @ wo.T) + 3*bv @ wo.T + 3*bo

v2 design (vs v1 baseline):
  - Weight-prep is sharded 8 ways: each core computes a 256-row d-slice of
    M~ = SQD*(wq.T @ wk) and of Wz = wv.T @ wo.T, then two AllGathers
    rebuild the full matrices on every core (~65k PE rows each instead of
    524k, gather ~50us hidden under compute).
  - Output projection fused: one matmul y = zv @ Wz + c0 instead of the
    two-step (wv then wo), with c0 = 3*bv @ wo.T + 3*bo.
  - Host pre-casts z and weights to bf16 (the on-chip matmuls consumed
    bf16 anyway) and pre-transposes wo; halves DMA and removes the big
    cast load from ACT/DVE.
  - r = bq @ wk and c0 are accumulated on ACT+DVE from the streamed
    weight chunks; only 6 of 9 score dots computed (mask kills 3).
  - zv^T stays in SBUF (no DRAM spill); Wz is streamed by e-quarters in
    the final y phase.
"""

import sys

sys.path.insert(0, "/opt/trn_rl_repo")

import ml_dtypes
import numpy as np
from concourse import bacc, bass, masks, mybir, tile
from concourse.bass_utils import run_bass_kernel_spmd

F32 = mybir.dt.float32
BF16 = mybir.dt.bfloat16
ADD = mybir.AluOpType.add
MULT = mybir.AluOpType.mult
CPY = mybir.ActivationFunctionType.Copy
EXP = mybir.ActivationFunctionType.Exp

B = 8192
D = 2048
T = 3
NCORES = 8
DC = D // 128      # 16 d-chunks
EC = D // 512      # 4 e-chunks (512-wide psum banks)
SH = D // NCORES   # 256 rows of M/Wz owned per core
SQD = 1.0 / float(np.sqrt(np.float32(D)))
BF = ml_dtypes.bfloat16


def emit(tc, aps, b_loc):
    nc = tc.nc
    z, wq_s, wk, wv_s, woT = aps["z"], aps["wq_s"], aps["wk"], aps["wv_s"], aps["woT"]
    bq, bk, bv, bo, out = aps["bq"], aps["bk"], aps["bv"], aps["bo"], aps["out"]
    NB = b_loc // 128

    const = tc.alloc_tile_pool(name="const", bufs=1)
    persist = tc.alloc_tile_pool(name="persist", bufs=1)

    ident = const.tile([128, 128], BF16)
    masks.make_identity(nc, ident[:])

    # bias columns: col[p, c] = vec[c*128 + p]
    bq_col = const.tile([128, DC], F32)
    bk_col = const.tile([128, DC], F32)
    bv_col = const.tile([128, DC], F32)
    bo_row = const.tile([1, D], F32)
    nc.sync.dma_start(bq_col[:], bq.rearrange("(c p) -> p c", p=128))
    nc.sync.dma_start(bk_col[:], bk.rearrange("(c p) -> p c", p=128))
    nc.sync.dma_start(bv_col[:], bv.rearrange("(c p) -> p c", p=128))
    nc.sync.dma_start(bo_row[:], bo[None, :])
    bq_colbf = const.tile([128, DC], BF16)
    bk_colbf = const.tile([128, DC], BF16)
    nc.vector.tensor_copy(bq_colbf[:], bq_col[:])
    nc.vector.tensor_copy(bk_colbf[:], bk_col[:])

    a_rep = persist.tile([128, D], BF16)    # SQD * wq.T @ bk, bcast
    r_rep = persist.tile([128, D], BF16)    # SQD * bq @ wk, bcast
    c0_rep = persist.tile([128, D], BF16)   # 3*bv @ wo.T + 3*bo, bcast
    kap_col = persist.tile([128, 1], F32)   # SQD * bq.bk

    # gather buffers (DRAM)
    ag1_in = nc.dram_tensor("ag1_in", [SH + 1, D], BF16).ap()
    ag1_out = nc.dram_tensor("ag1_out", [NCORES, SH + 1, D], BF16,
                             addr_space="Shared").ap()
    ag2_in = nc.dram_tensor("ag2_in", [SH, D], BF16).ap()
    ag2_out = nc.dram_tensor("ag2_out", [NCORES, SH, D], BF16,
                             addr_space="Shared").ap()

    zvT_pool = tc.alloc_tile_pool(name="zvT_pool", bufs=1, side="right")
    zvT_all = zvT_pool.tile([128, DC, b_loc], BF16)  # zv^T[d, b]
    m_pool = tc.alloc_tile_pool(name="m_pool", bufs=1, side="right")
    m_bf = m_pool.tile([128, DC, D], BF16)          # M~[d, e]

    # ---------------- Phase 0a: M~ slice + r + a + kap ---------------------
    with (
        tc.tile_pool(name="p0_w", bufs=1) as p_w,
        tc.tile_pool(name="p0_io", bufs=2) as p_io,
        tc.tile_pool(name="p0_acc", bufs=1) as p_acc,
        tc.tile_pool(name="p0_ps", bufs=1, space="PSUM") as pp,
    ):
        wq_sb = p_w.tile([128, DC, SH], BF16, tag="wq")
        nc.sync.dma_start(wq_sb[:], wq_s.rearrange("(c p) d -> p c d", p=128))
        ps_m = [pp.tile([128, 512], F32, tag=f"m{k}", name=f"ps_m{k}")
                for k in range(8)]
        racc = p_acc.tile([128, D], F32, tag="racc")
        for i in range(DC):
            wk_t = p_io.tile([128, D], BF16, tag="wkt", bufs=3)
            nc.scalar.dma_start(wk_t[:], wk[i * 128:(i + 1) * 128, :])
            for dd in range(2):
                for e in range(EC):
                    nc.tensor.matmul(
                        ps_m[dd * EC + e][:],
                        wq_sb[:, i, dd * 128:(dd + 1) * 128],
                        wk_t[:, e * 512:(e + 1) * 512],
                        start=(i == 0), stop=(i == DC - 1))
            # r += bq[i-chunk] * wk[i-chunk]  (ACT scaled copy + DVE add)
            if i == 0:
                nc.scalar.activation(racc[:], wk_t[:], CPY,
                                     scale=bq_col[:, 0:1])
            else:
                rt = p_io.tile([128, D], BF16, tag="rt", bufs=2)
                nc.scalar.activation(rt[:], wk_t[:], CPY,
                                     scale=bq_col[:, i:i + 1])
                nc.vector.tensor_tensor(racc[:], racc[:], rt[:], op=ADD)
        r_red = p_acc.tile([128, D], F32, tag="rred")
        nc.gpsimd.partition_all_reduce(
            r_red[:], racc[:], channels=128,
            reduce_op=bass.bass_isa.ReduceOp.add)
        nc.scalar.activation(r_rep[:], r_red[:], CPY, scale=SQD)

        # M~ slice -> stage -> ag1_in rows 1..256
        m_stage = p_acc.tile([128, 2, D], BF16, tag="stage")
        for dd in range(2):
            for e in range(EC):
                nc.scalar.activation(m_stage[:, dd, e * 512:(e + 1) * 512],
                                     ps_m[dd * EC + e][:], CPY, scale=SQD)
        nc.sync.dma_start(
            ag1_in[1:, :].rearrange("(dd p) d -> p dd d", p=128), m_stage[:])

        # a partial (own d-slice) on DVE: a[d] = sum_i wq[i, d] bk[i]
        aacc = p_acc.tile([128, SH], F32, tag="aacc")
        for i in range(DC):
            if i == 0:
                nc.vector.tensor_scalar(aacc[:], wq_sb[:, 0, :],
                                        bk_col[:, 0:1], None, op0=MULT)
            else:
                at = p_acc.tile([128, SH], BF16, tag="at", bufs=2)
                nc.vector.tensor_scalar(at[:], wq_sb[:, i, :],
                                        bk_col[:, i:i + 1], None, op0=MULT)
                nc.vector.tensor_tensor(aacc[:], aacc[:], at[:], op=ADD)
        a_red = p_acc.tile([128, SH], F32, tag="ared")
        nc.gpsimd.partition_all_reduce(
            a_red[:], aacc[:], channels=128,
            reduce_op=bass.bass_isa.ReduceOp.add)
        a_loc = p_acc.tile([1, SH], BF16, tag="aloc")
        nc.scalar.activation(a_loc[:], a_red[0:1, :], CPY, scale=SQD)
        nc.sync.dma_start(ag1_in[0:1, 0:SH], a_loc[:])

        # kap on DVE
        kt = p_acc.tile([128, DC], F32, tag="kt")
        nc.vector.tensor_tensor(kt[:], bq_col[:], bk_col[:], op=MULT)
        k1 = p_acc.tile([128, 1], F32, tag="k1")
        nc.vector.tensor_reduce(k1[:], kt[:], axis=mybir.AxisListType.X,
                                op=ADD)
        nc.gpsimd.partition_all_reduce(
            kap_col[:], k1[:], channels=128,
            reduce_op=bass.bass_isa.ReduceOp.add)
        nc.vector.tensor_scalar(kap_col[:], kap_col[:], SQD, None, op0=MULT)

        nc.gpsimd.collective_compute(
            "AllGather", mybir.AluOpType.bypass,
            replica_groups=[list(range(NCORES))],
            ins=[ag1_in], outs=[ag1_out])

        # ---------------- Phase 0b: Wz slice + c0 --------------------------
        wv_sb = p_w.tile([128, DC, SH], BF16, tag="wv")
        nc.sync.dma_start(wv_sb[:], wv_s.rearrange("(c p) d -> p c d", p=128))
        ps_z = [pp.tile([128, 512], F32, tag=f"m{k}", name=f"ps_z{k}")
                for k in range(8)]
        cacc = p_acc.tile([128, D], F32, tag="racc")
        for j in range(DC):
            wo_t = p_io.tile([128, D], BF16, tag="wkt", bufs=3)
            nc.scalar.dma_start(wo_t[:], woT[j * 128:(j + 1) * 128, :])
            for dd in range(2):
                for e in range(EC):
                    nc.tensor.matmul(
                        ps_z[dd * EC + e][:],
                        wv_sb[:, j, dd * 128:(dd + 1) * 128],
                        wo_t[:, e * 512:(e + 1) * 512],
                        start=(j == 0), stop=(j == DC - 1))
            if j == 0:
                nc.scalar.activation(cacc[:], wo_t[:], CPY,
                                     scale=bv_col[:, 0:1])
            else:
                ct = p_io.tile([128, D], BF16, tag="rt", bufs=2)
                nc.scalar.activation(ct[:], wo_t[:], CPY,
                                     scale=bv_col[:, j:j + 1])
                nc.vector.tensor_tensor(cacc[:], cacc[:], ct[:], op=ADD)
        # c0 = 3*(bv@woT) + 3*bo ; add 3*bo into partition 0 before reduce
        nc.vector.tensor_scalar(cacc[:], cacc[:], 3.0, None, op0=MULT)
        nc.vector.tensor_scalar(bo_row[:], bo_row[:], 3.0, None, op0=MULT)
        nc.vector.tensor_tensor(cacc[0:1, :], cacc[0:1, :], bo_row[:], op=ADD)
        c_red = p_acc.tile([128, D], F32, tag="rred")
        nc.gpsimd.partition_all_reduce(
            c_red[:], cacc[:], channels=128,
            reduce_op=bass.bass_isa.ReduceOp.add)
        nc.vector.tensor_copy(c0_rep[:], c_red[:])

        wz_stage = p_acc.tile([128, 2, D], BF16, tag="stage")
        for dd in range(2):
            for e in range(EC):
                nc.vector.tensor_copy(wz_stage[:, dd, e * 512:(e + 1) * 512],
                                      ps_z[dd * EC + e][:])
        nc.sync.dma_start(
            ag2_in.rearrange("(dd p) d -> p dd d", p=128), wz_stage[:])
        nc.gpsimd.collective_compute(
            "AllGather", mybir.AluOpType.bypass,
            replica_groups=[list(range(NCORES))],
            ins=[ag2_in], outs=[ag2_out])

        # load gathered M~ into SBUF; a row -> broadcast
        for dc in range(DC):
            c, h = dc // 2, dc % 2
            nc.scalar.dma_start(
                m_bf[:, dc, :], ag1_out[c, 1 + h * 128:1 + (h + 1) * 128, :])
        a_row = p_acc.tile([1, D], BF16, tag="arow")
        for c in range(NCORES):
            nc.sync.dma_start(a_row[0:1, c * SH:(c + 1) * SH],
                              ag1_out[c, 0:1, 0:SH])
        nc.gpsimd.partition_broadcast(a_rep[:], a_row[:])

    # ---------------- Phase 2: per b-tile scores/softmax/zv ----------------
    with (
        tc.tile_pool(name="p2_z", bufs=1) as p_z,
        tc.tile_pool(name="p2_g", bufs=1) as p_g,
        tc.tile_pool(name="p2_sc", bufs=1) as p_sc,
        tc.tile_pool(name="p2_io", bufs=1) as p_io,
        tc.tile_pool(name="p2_pst", bufs=2, space="PSUM") as pp_t,
        tc.tile_pool(name="p2_psg", bufs=5, space="PSUM") as pp_g,
    ):
        def sec_a(ib):
            """bf16 z loads + zq transposes for tile ib"""
            r0 = ib * 128
            st = {}
            st["zq"] = p_z.tile([128, T, D], BF16, tag="zq", bufs=2, name="zq")
            nc.sync.dma_start(st["zq"][:], z[r0:r0 + 128, 0:T * D])
            st["zk"] = p_z.tile([128, T, D], BF16, tag="zk", bufs=2, name="zk")
            nc.sync.dma_start(st["zk"][:], z[r0:r0 + 128, T * D:2 * T * D])
            st["zqT"] = p_z.tile([128, T, DC, 128], BF16, tag="zqT", bufs=1, name="zqT")
            for t in range(T):
                for dg in range(DC // 8):
                    ps = pp_t.tile([128, 8, 128], BF16)
                    for j in range(8):
                        d = dg * 8 + j
                        nc.tensor.matmul(
                            ps[:, j, :],
                            st["zq"][:, t, d * 128:(d + 1) * 128],
                            ident[:], is_transpose=True)
                    nc.scalar.activation(
                        st["zqT"][:, t, dg * 8:(dg + 1) * 8, :], ps[:], CPY)
            return st

        def sec_c(ib, st):
            """G~ = zq @ M~ on PE, fused with score dots per t (s >= t)"""
            sraw = p_sc.tile([128, T, T], F32, tag="sraw", bufs=2)
            st["sraw"] = sraw
            for t in range(T):
                gt = p_g.tile([128, D], BF16, tag="gt", bufs=2)
                for e in range(EC):
                    ps = pp_g.tile([128, 512], F32)
                    for d in range(DC):
                        nc.tensor.matmul(
                            ps[:], st["zqT"][:, t, d, :],
                            m_bf[:, d, e * 512:(e + 1) * 512],
                            start=(d == 0), stop=(d == DC - 1))
                    nc.scalar.activation(gt[:, e * 512:(e + 1) * 512],
                                         ps[:], CPY)
                for s in range(t, T):
                    scr = p_io.tile([128, D], BF16, tag="scr", bufs=2)
                    nc.vector.tensor_tensor(scr[:], gt[:],
                                            st["zk"][:, s, :], op=MULT)
                    if (t + s) % 2 == 0:
                        scr2 = p_io.tile([128, D], BF16, tag="scr", bufs=2)
                        nc.scalar.activation(scr2[:], scr[:], CPY,
                                             accum_out=sraw[:, t, s:s + 1])
                    else:
                        nc.vector.tensor_reduce(sraw[:, t, s:s + 1], scr[:],
                                                axis=mybir.AxisListType.X,
                                                op=ADD)

        def sec_b(ib, st):
            """a/r dots + softmax + zv (DVE/ACT only)"""
            sraw = st["sraw"]
            traw = p_sc.tile([128, T], F32, tag="traw", bufs=1)
            rzr = p_sc.tile([128, T], F32, tag="rzr", bufs=1)
            for t in range(T):
                scr = p_io.tile([128, D], BF16, tag="scr", bufs=2)
                nc.vector.tensor_tensor(scr[:], st["zq"][:, t, :], a_rep[:],
                                        op=MULT)
                if t == 1:
                    scr2 = p_io.tile([128, D], BF16, tag="scr", bufs=2)
                    nc.scalar.activation(scr2[:], scr[:], CPY,
                                         accum_out=traw[:, t:t + 1])
                else:
                    nc.vector.tensor_reduce(traw[:, t:t + 1], scr[:],
                                            axis=mybir.AxisListType.X, op=ADD)
            for s in range(T):
                scr = p_io.tile([128, D], BF16, tag="scr", bufs=2)
                nc.vector.tensor_tensor(scr[:], st["zk"][:, s, :], r_rep[:],
                                        op=MULT)
                if s == 1:
                    scr2 = p_io.tile([128, D], BF16, tag="scr", bufs=2)
                    nc.scalar.activation(scr2[:], scr[:], CPY,
                                         accum_out=rzr[:, s:s + 1])
                else:
                    nc.vector.tensor_reduce(rzr[:, s:s + 1], scr[:],
                                            axis=mybir.AxisListType.X, op=ADD)
            tvec = p_sc.tile([128, T], F32, tag="tvec", bufs=1)
            nc.vector.tensor_scalar(tvec[:], traw[:], 1.0, kap_col[:],
                                    op0=MULT, op1=ADD)
            # add the s-dependent r.zk term to the needed score entries
            for t in range(T):
                nc.vector.tensor_tensor(sraw[:, t, t:], sraw[:, t, t:],
                                        rzr[:, t:], op=ADD)
            # softmax; exp(score + tvec[t]); masked entries = exp(0) = 1
            p_un = p_sc.tile([128, T, T], F32, tag="p_un", bufs=1)
            nc.scalar.activation(p_un[:, 0, :], sraw[:, 0, :], EXP,
                                 bias=tvec[:, 0:1])
            nc.scalar.activation(p_un[:, 1, 1:], sraw[:, 1, 1:], EXP,
                                 bias=tvec[:, 1:2])
            nc.scalar.activation(p_un[:, 2, 2:], sraw[:, 2, 2:], EXP,
                                 bias=tvec[:, 2:3])
            nc.vector.memset(p_un[:, 1, 0:1], 1.0)
            nc.vector.memset(p_un[:, 2, 0:2], 1.0)
            rsum = p_sc.tile([128, T], F32, tag="rsum", bufs=1)
            nc.vector.tensor_reduce(rsum[:], p_un[:],
                                    axis=mybir.AxisListType.X, op=ADD)
            rinv = p_sc.tile([128, T], F32, tag="rinv", bufs=1)
            nc.vector.reciprocal(rinv[:], rsum[:])
            pn = p_sc.tile([128, T, T], F32, tag="pn", bufs=1)
            for t in range(T):
                nc.vector.tensor_scalar(pn[:, t, :], p_un[:, t, :],
                                        rinv[:, t:t + 1], None, op0=MULT)
            ws = p_sc.tile([128, T], F32, tag="ws", bufs=1)
            nc.vector.tensor_reduce(ws[:], pn.rearrange("p t s -> p s t"),
                                    axis=mybir.AxisListType.X, op=ADD)
            # zv = sum_s ws[s] * zq[s]
            zv_bf = p_sc.tile([128, D], BF16, tag="zv", bufs=2)
            zv_t1 = p_io.tile([128, D], BF16, tag="scr", bufs=2)
            nc.vector.tensor_scalar(zv_bf[:], st["zq"][:, 0, :], ws[:, 0:1],
                                    None, op0=MULT)
            nc.scalar.activation(zv_t1[:], st["zq"][:, 1, :], CPY,
                                 scale=ws[:, 1:2])
            nc.vector.tensor_tensor(zv_bf[:], zv_bf[:], zv_t1[:], op=ADD)
            nc.scalar.activation(zv_t1[:], st["zq"][:, 2, :], CPY,
                                 scale=ws[:, 2:3])
            nc.vector.tensor_tensor(zv_bf[:], zv_bf[:], zv_t1[:], op=ADD)
            st["zv"] = zv_bf

        def sec_d(ib, st):
            """transpose zv into the persistent zv^T[d, b] SBUF tensor"""
            for dg in range(DC // 8):
                ps = pp_t.tile([128, 8, 128], BF16)
                for j in range(8):
                    d = dg * 8 + j
                    nc.tensor.matmul(ps[:, j, :],
                                     st["zv"][:, d * 128:(d + 1) * 128],
                                     ident[:], is_transpose=True)
                nc.vector.tensor_copy(
                    zvT_all[:, dg * 8:(dg + 1) * 8, ib * 128:(ib + 1) * 128],
                    ps[:])

        state = [None] * NB
        for ib in range(NB):
            state[ib] = sec_a(ib)
            if ib > 0:
                sec_b(ib - 1, state[ib - 1])
            sec_c(ib, state[ib])
            if ib > 0:
                sec_d(ib - 1, state[ib - 1])
        sec_b(NB - 1, state[NB - 1])
        sec_d(NB - 1, state[NB - 1])

    m_pool.release()

    # ---------------- Phase 4: y = zv @ Wz + c0 ----------------------------
    with (
        tc.tile_pool(name="p4_wz", bufs=1) as p_wz,
        tc.tile_pool(name="p4_y", bufs=2) as p_y,
        tc.tile_pool(name="p4_ps", bufs=4, space="PSUM") as pp_y,
    ):
        for q in range(EC):
            wzq = p_wz.tile([128, DC, 512], BF16, tag="wzq", bufs=2)
            for dc in range(DC):
                c, h = dc // 2, dc % 2
                nc.scalar.dma_start(
                    wzq[:, dc, :],
                    ag2_out[c, h * 128:(h + 1) * 128,
                            q * 512:(q + 1) * 512])
            for ib in range(NB):
                ps = pp_y.tile([128, 512], F32)
                for dc in range(DC):
                    nc.tensor.matmul(
                        ps[:], zvT_all[:, dc, ib * 128:(ib + 1) * 128],
                        wzq[:, dc, :],
                        start=(dc == 0), stop=(dc == DC - 1))
                y_sb = p_y.tile([128, 512], F32)
                nc.vector.tensor_tensor(
                    y_sb[:], ps[:], c0_rep[:, q * 512:(q + 1) * 512], op=ADD)
                nc.sync.dma_start(
                    out[ib * 128:(ib + 1) * 128, q * 512:(q + 1) * 512],
                    y_sb[:])

    zvT_pool.release()
    persist.release()
    const.release()


def build_nc(b_loc):
    nc = bacc.Bacc("TRN2", target_bir_lowering=False, debug=False,
                   num_devices=NCORES)
    aps = {}
    aps["z"] = nc.dram_tensor("z", [b_loc, 2 * T * D], BF16,
                              kind="ExternalInput").ap()
    aps["wq_s"] = nc.dram_tensor("wq_s", [D, SH], BF16,
                                 kind="ExternalInput").ap()
    aps["wk"] = nc.dram_tensor("wk", [D, D], BF16, kind="ExternalInput").ap()
    aps["wv_s"] = nc.dram_tensor("wv_s", [D, SH], BF16,
                                 kind="ExternalInput").ap()
    aps["woT"] = nc.dram_tensor("woT", [D, D], BF16, kind="ExternalInput").ap()
    for b_ in ("bq", "bk", "bv", "bo"):
        aps[b_] = nc.dram_tensor(b_, [D], F32, kind="ExternalInput").ap()
    aps["out"] = nc.dram_tensor("out", [b_loc, D], F32,
                                kind="ExternalOutput").ap()
    with tile.TileContext(nc) as tc:
        emit(tc, aps, b_loc)
    nc.compile()
    return nc


_CACHE = {}


def _get_nc(b_loc):
    if b_loc not in _CACHE:
        _CACHE[b_loc] = build_nc(b_loc)
    return _CACHE[b_loc]


def make_in_maps(arrs):
    """Host-side sharding/layout prep: bf16 casts, wo transpose, slices."""
    b_loc = B // NCORES
    z_bf = np.ascontiguousarray(arrs["z"]).astype(BF)
    wk_bf = np.ascontiguousarray(arrs["wk"]).astype(BF)
    woT_bf = np.ascontiguousarray(arrs["wo"].T).astype(BF)
    biases = {k: np.ascontiguousarray(arrs[k], dtype=np.float32)
              for k in ("bq", "bk", "bv", "bo")}
    in_maps = []
    for c in range(NCORES):
        m = dict(biases)
        m["z"] = z_bf[c * b_loc:(c + 1) * b_loc]
        m["wk"] = wk_bf
        m["woT"] = woT_bf
        m["wq_s"] = np.ascontiguousarray(
            arrs["wq"][:, c * SH:(c + 1) * SH]).astype(BF)
        m["wv_s"] = np.ascontiguousarray(
            arrs["wv"][:, c * SH:(c + 1) * SH]).astype(BF)
        in_maps.append(m)
    return in_maps


def kernel(**inputs):
    arrs = {k: np.asarray(v) for k, v in inputs.items()}
    b_loc = B // NCORES
    nc = _get_nc(b_loc)
    in_maps = make_in_maps(arrs)
    res = run_bass_kernel_spmd(nc, in_maps, core_ids=list(range(NCORES)))
    return np.concatenate([np.asarray(r["out"]) for r in res.results], axis=0)


# revision 9
# speedup vs baseline: 1.2804x; 1.0049x over previous
"""Trainium2 Bass kernel for nn_Attention (sparse_attention, T=3) — v3.

Math (per batch row b, derived from the reference):
    zq = z[:, :3*2048].reshape(B, 3, D)   (q and v source)
    zk = z[:, 3*2048:].reshape(B, 3, D)
    scores[t,s] = (zq[t] @ (wq.T @ wk) @ zk[s] + zq[t].(wq.T @ bk)
                   + (bq @ wk).zk[s] + bq.bk) / sqrt(D)
    strictly-lower entries of scores are replaced by 0 before softmax
    p = softmax(scores, axis=-1); w[s] = sum_t p[t,s]  (sum_s w[s] == 3)
    y = (sum_s w[s]*zq[s]) @ (wv.T @ wo.T) + 3*bv @ wo.T + 3*bo

Design:
  - Weight-prep sharded 8 ways: each core computes a 256-row d-slice of
    M~ = SQD*(wq.T @ wk) and of Wz = wv.T @ wo.T; AllGathers rebuild the
    full matrices on every core. The M gather is split in two 128-row
    halves pipelined with compute: G starts on even d-chunks while odd
    chunks are still in flight.
  - Output projection fused: y = zv @ Wz + c0, c0 = 3*bv @ wo.T + 3*bo.
  - Host pre-casts z/weights to bf16 and pre-transposes wo.
  - r = bq @ wk, a-partial, c0, kap on ACT/DVE/GpSimd; 6 of 9 score dots.
  - zv^T stays in SBUF; Wz streamed by e-quarters in the y phase.
  - DMA spread over sync/scalar/vector/gpsimd queues.
"""

import sys

sys.path.insert(0, "/opt/trn_rl_repo")

import ml_dtypes
import numpy as np
from concourse import bacc, bass, masks, mybir, tile
from concourse.bass_utils import run_bass_kernel_spmd

F32 = mybir.dt.float32
BF16 = mybir.dt.bfloat16
ADD = mybir.AluOpType.add
MULT = mybir.AluOpType.mult
CPY = mybir.ActivationFunctionType.Copy
EXP = mybir.ActivationFunctionType.Exp
RADD = bass.bass_isa.ReduceOp.add

B = 8192
D = 2048
T = 3
NCORES = 8
DC = D // 128      # 16 d-chunks
EC = D // 512      # 4 e-chunks (512-wide psum banks)
SH = D // NCORES   # 256 rows of M/Wz owned per core
SQD = 1.0 / float(np.sqrt(np.float32(D)))
BF = ml_dtypes.bfloat16
# G accumulates even d-chunks (first gather half) before odd ones
D_ORDER = list(range(0, DC, 2)) + list(range(1, DC, 2))


def emit(tc, aps, b_loc):
    nc = tc.nc
    z, wq_s, wk, wv_s, woT = aps["z"], aps["wq_s"], aps["wk"], aps["wv_s"], aps["woT"]
    bq, bk, bv, bo, out = aps["bq"], aps["bk"], aps["bv"], aps["bo"], aps["out"]
    NB = b_loc // 128

    const = tc.alloc_tile_pool(name="const", bufs=1)
    persist = tc.alloc_tile_pool(name="persist", bufs=1)

    ident = const.tile([128, 128], BF16)
    masks.make_identity(nc, ident[:])

    # bias columns: col[p, c] = vec[c*128 + p]
    bq_col = const.tile([128, DC], F32)
    bk_col = const.tile([128, DC], F32)
    bv_col = const.tile([128, DC], F32)
    bo_row = const.tile([1, D], F32)
    nc.sync.dma_start(bq_col[:], bq.rearrange("(c p) -> p c", p=128))
    nc.sync.dma_start(bk_col[:], bk.rearrange("(c p) -> p c", p=128))
    nc.sync.dma_start(bv_col[:], bv.rearrange("(c p) -> p c", p=128))
    nc.sync.dma_start(bo_row[:], bo[None, :])

    a_rep = persist.tile([128, D], BF16)    # SQD * wq.T @ bk, bcast
    r_rep = persist.tile([128, D], BF16)    # SQD * bq @ wk, bcast
    c0_rep = persist.tile([128, D], BF16)   # 3*bv @ wo.T + 3*bo, bcast
    kap_col = persist.tile([128, 1], F32)   # SQD * bq.bk
    a_row = persist.tile([1, D], BF16)

    # gather buffers (DRAM); M gather split into two 128-row halves
    ag1a_in = nc.dram_tensor("ag1a_in", [129, D], BF16).ap()
    ag1a_out = nc.dram_tensor("ag1a_out", [NCORES, 129, D], BF16,
                              addr_space="Shared").ap()
    ag1b_in = nc.dram_tensor("ag1b_in", [128, D], BF16).ap()
    ag1b_out = nc.dram_tensor("ag1b_out", [NCORES, 128, D], BF16,
                              addr_space="Shared").ap()
    ag2_in = nc.dram_tensor("ag2_in", [SH, D], BF16).ap()
    ag2_out = nc.dram_tensor("ag2_out", [NCORES, SH, D], BF16,
                             addr_space="Shared").ap()

    zvT_pool = tc.alloc_tile_pool(name="zvT_pool", bufs=1, side="right")
    zvT_all = zvT_pool.tile([128, DC, b_loc], BF16)  # zv^T[d, b]

    RG = [list(range(NCORES))]

    # ---------------- Phase 0: M~/Wz slices + r/a/c0/kap -------------------
    with (
        tc.tile_pool(name="p0_w", bufs=1) as p_w,
        tc.tile_pool(name="p0_io", bufs=2) as p_io,
        tc.tile_pool(name="p0_acc", bufs=1) as p_acc,
        tc.tile_pool(name="p0_ps", bufs=1, space="PSUM") as pp,
    ):
        # chunked contiguous weight loads, spread over queues
        wq_sb = p_w.tile([128, DC, SH], BF16, tag="wq")
        wk_sb = p_w.tile([128, DC, D], BF16, tag="wk")
        wv_sb = p_w.tile([128, DC, SH], BF16, tag="wv")
        nc.sync.dma_start(wq_sb[:, 0, :], wq_s[0:128, :])
        nc.scalar.dma_start(wk_sb[:, 0, :], wk[0:128, :])
        for i in range(1, DC):
            eng = nc.scalar if i % 2 == 0 else nc.sync
            eng.dma_start(wk_sb[:, i, :], wk[i * 128:(i + 1) * 128, :])
            nc.sync.dma_start(wq_sb[:, i, :], wq_s[i * 128:(i + 1) * 128, :])
        nc.gpsimd.dma_start(wv_sb[:], wv_s.rearrange("(c p) d -> p c d", p=128))

        ps_m = [pp.tile([128, 512], F32, tag=f"m{k}", name=f"ps_m{k}")
                for k in range(8)]

        # a partial (own d-slice) on DVE: a[d] = sum_i wq[i, d] bk[i]
        aacc = p_acc.tile([128, SH], F32, tag="aacc")
        for i in range(DC):
            if i == 0:
                nc.vector.tensor_scalar(aacc[:], wq_sb[:, 0, :],
                                        bk_col[:, 0:1], None, op0=MULT)
            else:
                at = p_acc.tile([128, SH], BF16, tag="at", bufs=2)
                nc.vector.tensor_scalar(at[:], wq_sb[:, i, :],
                                        bk_col[:, i:i + 1], None, op0=MULT)
                nc.vector.tensor_tensor(aacc[:], aacc[:], at[:], op=ADD)
        a_red = p_acc.tile([128, SH], F32, tag="ared")
        nc.gpsimd.partition_all_reduce(a_red[:], aacc[:], channels=128,
                                       reduce_op=RADD)
        a_loc = p_acc.tile([1, SH], BF16, tag="aloc")
        nc.scalar.activation(a_loc[:], a_red[0:1, :], CPY, scale=SQD)
        nc.sync.dma_start(ag1a_in[0:1, 0:SH], a_loc[:])

        # M~ slice in two 128-row halves, each gathered separately
        for dd in range(2):
            for i in range(DC):
                for e in range(EC):
                    nc.tensor.matmul(
                        ps_m[dd * EC + e][:],
                        wq_sb[:, i, dd * 128:(dd + 1) * 128],
                        wk_sb[:, i, e * 512:(e + 1) * 512],
                        start=(i == 0), stop=(i == DC - 1))
            m_stage = p_acc.tile([128, D], BF16, tag="stage", bufs=2,
                                 name="m_stage")
            for e in range(EC):
                nc.scalar.activation(m_stage[:, e * 512:(e + 1) * 512],
                                     ps_m[dd * EC + e][:], CPY, scale=SQD)
            if dd == 0:
                nc.sync.dma_start(ag1a_in[1:129, :], m_stage[:])
                nc.gpsimd.collective_compute(
                    "AllGather", mybir.AluOpType.bypass, replica_groups=RG,
                    ins=[ag1a_in], outs=[ag1a_out])
            else:
                nc.sync.dma_start(ag1b_in[:, :], m_stage[:])
                nc.gpsimd.collective_compute(
                    "AllGather", mybir.AluOpType.bypass, replica_groups=RG,
                    ins=[ag1b_in], outs=[ag1b_out])

        # kap on DVE
        kt = p_acc.tile([128, DC], F32, tag="kt")
        nc.vector.tensor_tensor(kt[:], bq_col[:], bk_col[:], op=MULT)
        k1 = p_acc.tile([128, 1], F32, tag="k1")
        nc.vector.tensor_reduce(k1[:], kt[:], axis=mybir.AxisListType.X,
                                op=ADD)
        nc.gpsimd.partition_all_reduce(kap_col[:], k1[:], channels=128,
                                       reduce_op=RADD)
        nc.vector.tensor_scalar(kap_col[:], kap_col[:], SQD, None, op0=MULT)

        # ---- Wz slice + c0 ----
        ps_z = [pp.tile([128, 512], F32, tag=f"m{k}", name=f"ps_z{k}")
                for k in range(8)]
        cacc = p_acc.tile([128, D], F32, tag="racc")
        for j in range(DC):
            wo_t = p_io.tile([128, D], BF16, tag="wot", bufs=3)
            eng = nc.scalar if j % 2 == 0 else nc.sync
            eng.dma_start(wo_t[:], woT[j * 128:(j + 1) * 128, :])
            for dd in range(2):
                for e in range(EC):
                    nc.tensor.matmul(
                        ps_z[dd * EC + e][:],
                        wv_sb[:, j, dd * 128:(dd + 1) * 128],
                        wo_t[:, e * 512:(e + 1) * 512],
                        start=(j == 0), stop=(j == DC - 1))
            if j == 0:
                nc.scalar.activation(cacc[:], wo_t[:], CPY,
                                     scale=bv_col[:, 0:1])
            else:
                ct = p_io.tile([128, D], BF16, tag="ct", bufs=2)
                nc.scalar.activation(ct[:], wo_t[:], CPY,
                                     scale=bv_col[:, j:j + 1])
                nc.vector.tensor_tensor(cacc[:], cacc[:], ct[:], op=ADD)
        wz_stage = p_acc.tile([128, 2, D], BF16, tag="wzstage")
        for dd in range(2):
            for e in range(EC):
                nc.scalar.activation(wz_stage[:, dd, e * 512:(e + 1) * 512],
                                     ps_z[dd * EC + e][:], CPY)
        nc.sync.dma_start(
            ag2_in.rearrange("(dd p) d -> p dd d", p=128), wz_stage[:])
        nc.gpsimd.collective_compute(
            "AllGather", mybir.AluOpType.bypass, replica_groups=RG,
            ins=[ag2_in], outs=[ag2_out])

        # c0 = 3*(bv@woT) + 3*bo ; add 3*bo into partition 0 before reduce
        nc.vector.tensor_scalar(cacc[:], cacc[:], 3.0, None, op0=MULT)
        nc.vector.tensor_scalar(bo_row[:], bo_row[:], 3.0, None, op0=MULT)
        nc.vector.tensor_tensor(cacc[0:1, :], cacc[0:1, :], bo_row[:], op=ADD)
        c_red = p_acc.tile([128, D], F32, tag="cred")
        nc.gpsimd.partition_all_reduce(c_red[:], cacc[:], channels=128,
                                       reduce_op=RADD)
        nc.vector.tensor_copy(c0_rep[:], c_red[:])

        # r = bq @ wk on ACT+DVE from the resident wk chunks
        racc = p_acc.tile([128, D], F32, tag="racc2")
        for i in range(DC):
            if i == 0:
                nc.scalar.activation(racc[:], wk_sb[:, 0, :], CPY,
                                     scale=bq_col[:, 0:1])
            else:
                rt = p_io.tile([128, D], BF16, tag="ct", bufs=2)
                nc.scalar.activation(rt[:], wk_sb[:, i, :], CPY,
                                     scale=bq_col[:, i:i + 1])
                nc.vector.tensor_tensor(racc[:], racc[:], rt[:], op=ADD)
        r_red = p_acc.tile([128, D], F32, tag="cred")
        nc.gpsimd.partition_all_reduce(r_red[:], racc[:], channels=128,
                                       reduce_op=RADD)
        nc.scalar.activation(r_rep[:], r_red[:], CPY, scale=SQD)

        # a row from first gather half -> broadcast
        for c in range(NCORES):
            nc.gpsimd.dma_start(a_row[0:1, c * SH:(c + 1) * SH],
                                ag1a_out[c, 0:1, 0:SH])
        nc.gpsimd.partition_broadcast(a_rep[:], a_row[:])

    # gathered M~ into SBUF: even chunks from half a, odd from half b
    m_pool = tc.alloc_tile_pool(name="m_pool", bufs=1, side="right")
    m_bf = m_pool.tile([128, DC, D], BF16)          # M~[d, e]
    for c in range(NCORES):
        nc.gpsimd.dma_start(m_bf[:, 2 * c, :], ag1a_out[c, 1:129, :])
    for c in range(NCORES):
        nc.gpsimd.dma_start(m_bf[:, 2 * c + 1, :], ag1b_out[c, :, :])

    # ---------------- Phase 2: per b-tile scores/softmax/zv ----------------
    with (
        tc.tile_pool(name="p2_z", bufs=1) as p_z,
        tc.tile_pool(name="p2_g", bufs=1) as p_g,
        tc.tile_pool(name="p2_sc", bufs=1) as p_sc,
        tc.tile_pool(name="p2_io", bufs=1) as p_io,
        tc.tile_pool(name="p2_pst", bufs=2, space="PSUM") as pp_t,
        tc.tile_pool(name="p2_psg", bufs=5, space="PSUM") as pp_g,
    ):
        def sec_a(ib):
            """bf16 z loads + zq transposes for tile ib"""
            r0 = ib * 128
            st = {}
            st["zq"] = p_z.tile([128, T, D], BF16, tag="zq", bufs=2, name="zq")
            nc.sync.dma_start(st["zq"][:], z[r0:r0 + 128, 0:T * D])
            st["zk"] = p_z.tile([128, T, D], BF16, tag="zk", bufs=2, name="zk")
            nc.scalar.dma_start(st["zk"][:], z[r0:r0 + 128, T * D:2 * T * D])
            st["zqT"] = p_z.tile([128, T, DC, 128], BF16, tag="zqT", bufs=1,
                                 name="zqT")
            for t in range(T):
                for dg in range(DC // 8):
                    ps = pp_t.tile([128, 8, 128], BF16)
                    for j in range(8):
                        d = dg * 8 + j
                        nc.tensor.matmul(
                            ps[:, j, :],
                            st["zq"][:, t, d * 128:(d + 1) * 128],
                            ident[:], is_transpose=True)
                    nc.scalar.activation(
                        st["zqT"][:, t, dg * 8:(dg + 1) * 8, :], ps[:], CPY)
            return st

        def sec_c(ib, st):
            """G~ = zq @ M~ on PE, fused with score dots per t (s >= t)"""
            sraw = p_sc.tile([128, T, T], F32, tag="sraw", bufs=2)
            st["sraw"] = sraw
            for t in range(T):
                gt = p_g.tile([128, D], BF16, tag="gt", bufs=2)
                for e in range(EC):
                    ps = pp_g.tile([128, 512], F32)
                    for k, d in enumerate(D_ORDER):
                        nc.tensor.matmul(
                            ps[:], st["zqT"][:, t, d, :],
                            m_bf[:, d, e * 512:(e + 1) * 512],
                            start=(k == 0), stop=(k == DC - 1))
                    nc.scalar.activation(gt[:, e * 512:(e + 1) * 512],
                                         ps[:], CPY)
                for s in range(t, T):
                    scr = p_io.tile([128, D], BF16, tag="scr", bufs=2)
                    nc.vector.tensor_tensor(scr[:], gt[:],
                                            st["zk"][:, s, :], op=MULT)
                    if (t + s) % 2 == 0:
                        scr2 = p_io.tile([128, D], BF16, tag="scr", bufs=2)
                        nc.scalar.activation(scr2[:], scr[:], CPY,
                                             accum_out=sraw[:, t, s:s + 1])
                    else:
                        nc.vector.tensor_reduce(sraw[:, t, s:s + 1], scr[:],
                                                axis=mybir.AxisListType.X,
                                                op=ADD)

        def sec_b(ib, st):
            """a/r dots + softmax + zv (DVE/ACT only)"""
            sraw = st["sraw"]
            traw = p_sc.tile([128, T], F32, tag="traw", bufs=1)
            rzr = p_sc.tile([128, T], F32, tag="rzr", bufs=1)
            for t in range(T):
                scr = p_io.tile([128, D], BF16, tag="scr", bufs=2)
                nc.vector.tensor_tensor(scr[:], st["zq"][:, t, :], a_rep[:],
                                        op=MULT)
                if t == 1:
                    scr2 = p_io.tile([128, D], BF16, tag="scr", bufs=2)
                    nc.scalar.activation(scr2[:], scr[:], CPY,
                                         accum_out=traw[:, t:t + 1])
                else:
                    nc.vector.tensor_reduce(traw[:, t:t + 1], scr[:],
                                            axis=mybir.AxisListType.X, op=ADD)
            for s in range(T):
                scr = p_io.tile([128, D], BF16, tag="scr", bufs=2)
                nc.vector.tensor_tensor(scr[:], st["zk"][:, s, :], r_rep[:],
                                        op=MULT)
                if s == 1:
                    scr2 = p_io.tile([128, D], BF16, tag="scr", bufs=2)
                    nc.scalar.activation(scr2[:], scr[:], CPY,
                                         accum_out=rzr[:, s:s + 1])
                else:
                    nc.vector.tensor_reduce(rzr[:, s:s + 1], scr[:],
                                            axis=mybir.AxisListType.X, op=ADD)
            tvec = p_sc.tile([128, T], F32, tag="tvec", bufs=1)
            nc.vector.tensor_scalar(tvec[:], traw[:], 1.0, kap_col[:],
                                    op0=MULT, op1=ADD)
            # add the s-dependent r.zk term to the needed score entries
            for t in range(T):
                nc.vector.tensor_tensor(sraw[:, t, t:], sraw[:, t, t:],
                                        rzr[:, t:], op=ADD)
            # softmax; exp(score + tvec[t]); masked entries = exp(0) = 1
            p_un = p_sc.tile([128, T, T], F32, tag="p_un", bufs=1)
            nc.scalar.activation(p_un[:, 0, :], sraw[:, 0, :], EXP,
                                 bias=tvec[:, 0:1])
            nc.scalar.activation(p_un[:, 1, 1:], sraw[:, 1, 1:], EXP,
                                 bias=tvec[:, 1:2])
            nc.scalar.activation(p_un[:, 2, 2:], sraw[:, 2, 2:], EXP,
                                 bias=tvec[:, 2:3])
            nc.vector.memset(p_un[:, 1, 0:1], 1.0)
            nc.vector.memset(p_un[:, 2, 0:2], 1.0)
            rsum = p_sc.tile([128, T], F32, tag="rsum", bufs=1)
            nc.vector.tensor_reduce(rsum[:], p_un[:],
                                    axis=mybir.AxisListType.X, op=ADD)
            rinv = p_sc.tile([128, T], F32, tag="rinv", bufs=1)
            nc.vector.reciprocal(rinv[:], rsum[:])
            pn = p_sc.tile([128, T, T], F32, tag="pn", bufs=1)
            for t in range(T):
                nc.vector.tensor_scalar(pn[:, t, :], p_un[:, t, :],
                                        rinv[:, t:t + 1], None, op0=MULT)
            ws = p_sc.tile([128, T], F32, tag="ws", bufs=1)
            nc.vector.tensor_reduce(ws[:], pn.rearrange("p t s -> p s t"),
                                    axis=mybir.AxisListType.X, op=ADD)
            # zv = sum_s ws[s] * zq[s]
            zv_bf = p_sc.tile([128, D], BF16, tag="zv", bufs=2)
            zv_t1 = p_io.tile([128, D], BF16, tag="scr", bufs=2)
            nc.vector.tensor_scalar(zv_bf[:], st["zq"][:, 0, :], ws[:, 0:1],
                                    None, op0=MULT)
            nc.scalar.activation(zv_t1[:], st["zq"][:, 1, :], CPY,
                                 scale=ws[:, 1:2])
            nc.vector.tensor_tensor(zv_bf[:], zv_bf[:], zv_t1[:], op=ADD)
            nc.scalar.activation(zv_t1[:], st["zq"][:, 2, :], CPY,
                                 scale=ws[:, 2:3])
            nc.vector.tensor_tensor(zv_bf[:], zv_bf[:], zv_t1[:], op=ADD)
            st["zv"] = zv_bf

        def sec_d(ib, st):
            """transpose zv into the persistent zv^T[d, b] SBUF tensor"""
            for dg in range(DC // 8):
                ps = pp_t.tile([128, 8, 128], BF16)
                for j in range(8):
                    d = dg * 8 + j
                    nc.tensor.matmul(ps[:, j, :],
                                     st["zv"][:, d * 128:(d + 1) * 128],
                                     ident[:], is_transpose=True)
                nc.vector.tensor_copy(
                    zvT_all[:, dg * 8:(dg + 1) * 8, ib * 128:(ib + 1) * 128],
                    ps[:])

        state = [None] * NB
        for ib in range(NB):
            state[ib] = sec_a(ib)
            if ib > 0:
                sec_b(ib - 1, state[ib - 1])
            sec_c(ib, state[ib])
            if ib > 0:
                sec_d(ib - 1, state[ib - 1])
        sec_b(NB - 1, state[NB - 1])
        sec_d(NB - 1, state[NB - 1])

    m_pool.release()

    # ---------------- Phase 4: y = zv @ Wz + c0 ----------------------------
    p_wz = tc.alloc_tile_pool(name="p4_wz", bufs=1, side="right")
    with (
        tc.tile_pool(name="p4_y", bufs=2) as p_y,
        tc.tile_pool(name="p4_ps", bufs=4, space="PSUM") as pp_y,
    ):
        for q in range(EC):
            wzq = p_wz.tile([128, DC, 512], BF16, tag="wzq", bufs=2)
            for dc in range(DC):
                c, h = dc // 2, dc % 2
                eng = nc.gpsimd if dc % 2 == 0 else nc.scalar
                eng.dma_start(
                    wzq[:, dc, :],
                    ag2_out[c, h * 128:(h + 1) * 128, q * 512:(q + 1) * 512])
            for ib in range(NB):
                ps = pp_y.tile([128, 512], F32)
                for dc in range(DC):
                    nc.tensor.matmul(
                        ps[:], zvT_all[:, dc, ib * 128:(ib + 1) * 128],
                        wzq[:, dc, :],
                        start=(dc == 0), stop=(dc == DC - 1))
                y_sb = p_y.tile([128, 512], F32)
                nc.vector.tensor_tensor(
                    y_sb[:], ps[:], c0_rep[:, q * 512:(q + 1) * 512], op=ADD)
                nc.sync.dma_start(
                    out[ib * 128:(ib + 1) * 128, q * 512:(q + 1) * 512],
                    y_sb[:])

    p_wz.release()
    zvT_pool.release()
    persist.release()
    const.release()


def build_nc(b_loc):
    nc = bacc.Bacc("TRN2", target_bir_lowering=False, debug=False,
                   num_devices=NCORES)
    aps = {}
    aps["z"] = nc.dram_tensor("z", [b_loc, 2 * T * D], BF16,
                              kind="ExternalInput").ap()
    aps["wq_s"] = nc.dram_tensor("wq_s", [D, SH], BF16,
                                 kind="ExternalInput").ap()
    aps["wk"] = nc.dram_tensor("wk", [D, D], BF16, kind="ExternalInput").ap()
    aps["wv_s"] = nc.dram_tensor("wv_s", [D, SH], BF16,
                                 kind="ExternalInput").ap()
    aps["woT"] = nc.dram_tensor("woT", [D, D], BF16, kind="ExternalInput").ap()
    for b_ in ("bq", "bk", "bv", "bo"):
        aps[b_] = nc.dram_tensor(b_, [D], F32, kind="ExternalInput").ap()
    aps["out"] = nc.dram_tensor("out", [b_loc, D], F32,
                                kind="ExternalOutput").ap()
    with tile.TileContext(nc) as tc:
        emit(tc, aps, b_loc)
    nc.compile()
    return nc


_CACHE = {}


def _get_nc(b_loc):
    if b_loc not in _CACHE:
        _CACHE[b_loc] = build_nc(b_loc)
    return _CACHE[b_loc]


def make_in_maps(arrs):
    """Host-side sharding/layout prep: bf16 casts, wo transpose, slices."""
    b_loc = B // NCORES
    z_bf = np.ascontiguousarray(arrs["z"]).astype(BF)
    wk_bf = np.ascontiguousarray(arrs["wk"]).astype(BF)
    woT_bf = np.ascontiguousarray(arrs["wo"].T).astype(BF)
    biases = {k: np.ascontiguousarray(arrs[k], dtype=np.float32)
              for k in ("bq", "bk", "bv", "bo")}
    in_maps = []
    for c in range(NCORES):
        m = dict(biases)
        m["z"] = z_bf[c * b_loc:(c + 1) * b_loc]
        m["wk"] = wk_bf
        m["woT"] = woT_bf
        m["wq_s"] = np.ascontiguousarray(
            arrs["wq"][:, c * SH:(c + 1) * SH]).astype(BF)
        m["wv_s"] = np.ascontiguousarray(
            arrs["wv"][:, c * SH:(c + 1) * SH]).astype(BF)
        in_maps.append(m)
    return in_maps


def kernel(**inputs):
    arrs = {k: np.asarray(v) for k, v in inputs.items()}
    b_loc = B // NCORES
    nc = _get_nc(b_loc)
    in_maps = make_in_maps(arrs)
    res = run_bass_kernel_spmd(nc, in_maps, core_ids=list(range(NCORES)))
    return np.concatenate([np.asarray(r["out"]) for r in res.results], axis=0)
